# revision 24
# baseline (speedup 1.0000x reference)
"""H2GCN forward on 8 Trainium2 NeuronCores.

out = concat([h0, A1@h0, A2@h0], 1) @ W_out + b_out,  h0 = x @ W1

Data-parallel over destination nodes (1250 rows/core, padded to 1280).
v3 layout:
  - phase A: h0 = x @ W1 in bf16, k-outer loop over resident xt chunks,
    tiles 0-5 finished first (6 then 4 PSUM accumulators).
  - AllGather of h0 in fp8, split in two (tiles 0-5, 6-9 of every core)
    so SpMM starts on first-half source pairs while the second half is
    still on the wire.  (The collective subsystem has a ~77us boot
    barrier; both AGs queue right behind it.)
  - SpMM flipped: h1^T/h2^T = h0^T A with h0 fp8 pairs as the stationary
    operand (DoubleRow: 256 src rows per matmul) and dense fp8 A^T blocks
    as the moving operand, accumulated over 40 src-tile pairs into 6 PSUM
    banks (2 feature halves x 3 dst chunks).  Edge values pre-scaled
    x16/x32 into fp8 range; compensated in W_out rows.
  - h0 transposes (20) run under the AG window; h1/h2 need no transpose.
  - out = hT @ W_out + b: h0 contribution in fp32r, h1/h2 in bf16.
"""
import sys
import types

for _p in ("/opt/trn_rl_repo", "/root/.axon_site", "/root/.axon_site/_ro/trn_rl_repo",
           "/root/.axon_site/_ro/pypackages"):
    if _p not in sys.path:
        sys.path.append(_p)

import numpy as np
import ml_dtypes
import concourse.bass as bass
import concourse.bacc as bacc
import concourse.mybir as mybir
import concourse.tile as tile
from concourse import bass_utils

N, IN_C, HID, OUT_C = 10000, 2048, 256, 256
NCORES = 8
ROWS = N // NCORES          # 1250
PROWS = 1280                # padded (10 x 128)
NT = PROWS // 128           # 10 dst tiles
KT = IN_C // 128            # 16 k tiles
ST = NCORES * NT            # 80 src tiles in padded AllGather space
SP = ST // 2                # 40 src-tile pairs (DoubleRow)
CH = [(0, 512), (512, 512), (1024, 226)]   # dst chunks (pad rows trimmed)
AGS = [(0, 4), (4, 4), (8, 2)]  # AllGather thirds (tile start, count)

f32 = mybir.dt.float32
f32r = mybir.dt.float32r
bf16 = mybir.dt.bfloat16
f8 = mybir.dt.float8e4
bfnp = ml_dtypes.bfloat16
f8np = ml_dtypes.float8_e4m3

A1_SCALE = 16.0
A2_SCALE = 32.0

# wo8 layout (bf16 elems): Wout k-tiles 0-5 | bias (row 0) | ones (row 0)
WO8, OB8 = 0, 6 * OUT_C
OO8 = OB8 + OUT_C
WO8N = OO8 + 128

# SpMM pair order follows the AllGather thirds: pairs j 0-1 of each core,
# then j 2-3, then j 4
PAIR_ORDER = [r * 5 + j for r in range(NCORES) for j in (0, 1)] + \
             [r * 5 + j for r in range(NCORES) for j in (2, 3)] + \
             [r * 5 + 4 for r in range(NCORES)]

LAST_EXEC_NS = None
LAST_RESULTS = None


def _install_trace_shim():
    try:
        import antenv.axon_hooks  # noqa: F401
        return
    except ImportError:
        pass
    try:
        import antenv
        from trn_agent_boot.trn_boot import _ntff_profile_via_ctypes
        hook = _ntff_profile_via_ctypes("/opt/axon/libaxon_pjrt.so")
        mod = types.ModuleType("antenv.axon_hooks")
        mod.get_axon_ntff_profile_hook = lambda: hook
        mod.set_axon_ntff_profile_hook = lambda h: None
        sys.modules["antenv.axon_hooks"] = mod
        antenv.axon_hooks = mod
    except Exception:
        pass


def _dense_adj(rows, cols, vals, core, scale):
    """Dense padded A^T for this core's dest shard, src-tile-major:
    [128, ST*PROWS] fp8 with src tile s at columns [s*1280, (s+1)*1280)."""
    lo, hi = core * ROWS, (core + 1) * ROWS
    m = (rows >= lo) & (rows < hi)
    r, c, v = rows[m] - lo, cols[m], vals[m] * scale
    A = np.zeros((NCORES * PROWS, PROWS), np.float32)
    src = (c // ROWS) * PROWS + (c % ROWS)
    np.add.at(A, (src, r), v)
    return np.ascontiguousarray(
        A.reshape(ST, 128, PROWS).transpose(1, 0, 2)
        .reshape(128, ST * PROWS)).astype(f8np)


def _build():
    nc = bacc.Bacc("TRN2", target_bir_lowering=False, debug=False,
                   num_devices=8)
    w1_d = nc.dram_tensor("w1", [128, KT * HID], bf16, kind="ExternalInput")
    xt_d = nc.dram_tensor("xt", [128, KT * PROWS], bf16, kind="ExternalInput")
    ident_d = nc.dram_tensor("ident", [128, 128], f32, kind="ExternalInput")
    wo8_d = nc.dram_tensor("wo8", [128, WO8N], bf16, kind="ExternalInput")
    A1 = nc.dram_tensor("A1", [128, ST * PROWS], f8, kind="ExternalInput")
    A2 = nc.dram_tensor("A2", [128, ST * PROWS], f8, kind="ExternalInput")
    out = nc.dram_tensor("out", [ROWS, OUT_C], f32, kind="ExternalOutput")

    with tile.TileContext(nc) as tc:
        with tc.tile_pool(name="keep", bufs=1) as keep, \
             tc.tile_pool(name="dram", bufs=1, space="DRAM") as dram, \
             tc.tile_pool(name="pT", bufs=1, space="PSUM") as pT:

            h0_sb = keep.tile([128, NT, HID], f32)
            ag_sb = keep.tile([128, NT, HID], f8)
            h0a8 = keep.tile([128, ST, HID], f8)
            hT8 = keep.tile([128, 6, PROWS], bf16)
            wo8_sb = keep.tile([128, WO8N], bf16)
            ident_v = keep.tile([128, 128], f32)
            w1_sb = keep.tile([128, KT, HID], bf16)
            nc.sync.dma_start(w1_sb[:], w1_d[:].rearrange(
                "p (k m) -> p k m", k=KT))
            nc.sync.dma_start(wo8_sb[:], wo8_d[:])
            ident_t = keep.tile([128, 128], f32)
            nc.sync.dma_start(ident_t[:], ident_d[:])
            # identity produced on DVE so transposes need only one DVE wait
            nc.vector.tensor_copy(ident_v[:], ident_t[:])

            ag_ins, ag_outs = [], []
            for gi, (glo, gn) in enumerate(AGS):
                ag_ins.append(dram.tile([gn * 128, HID], f8,
                                        name=f"ag_in{gi}"))
                ag_outs.append(dram.tile([NCORES * gn * 128, HID], f8,
                                         addr_space="Shared",
                                         name=f"ag_out{gi}"))

            # ---- phase A: h0 = x @ W1 (bf16), k-outer so DMA pipelines.
            # Tiles 0-5 first so the first AllGather half can launch early.
            with nc.named_scope("h0_gemm"):
                with tc.tile_pool(name="pa", bufs=1, space="PSUM") as pa, \
                     tc.tile_pool(name="px", bufs=1) as px:
                    xts = []
                    for k in range(KT):
                        xt_k = px.tile([128, PROWS], bf16, tag=f"xt{k}",
                                       name=f"xt{k}")
                        nc.sync.dma_start(xt_k[:],
                                          xt_d[:, k * PROWS:(k + 1) * PROWS])
                        xts.append(xt_k)
                    for tlo, tn in ((0, 5), (5, 5)):
                        psA = [pa.tile([128, HID], f32, tag=f"a{i}",
                                       name=f"psA{i}") for i in range(tn)]
                        for k in range(KT):
                            for i in range(tn):
                                t = tlo + i
                                nc.tensor.matmul(
                                    psA[i][:],
                                    xts[k][:, 128 * t:128 * (t + 1)],
                                    w1_sb[:, k, :],
                                    start=(k == 0), stop=(k == KT - 1),
                                )
                        for i in range(tn):
                            t = tlo + i
                            nc.vector.tensor_copy(h0_sb[:, t, :], psA[i][:])
                            nc.vector.tensor_copy(ag_sb[:, t, :], psA[i][:])
                    for gi, (glo, gn) in enumerate(AGS):
                        nc.sync.dma_start(
                            ag_ins[gi][:].rearrange("(a p) m -> p a m", p=128),
                            ag_sb[:, glo:glo + gn, :])

            # ---- phase B: AllGather h0 (fp8), three chunks ----
            with nc.named_scope("allgather"):
                for gi in range(len(AGS)):
                    nc.gpsimd.collective_compute(
                        "AllGather", mybir.AluOpType.bypass,
                        replica_groups=[list(range(NCORES))],
                        ins=[ag_ins[gi].opt()], outs=[ag_outs[gi].opt()],
                    )

            # ---- phase C: transpose h0 -> feature-major (fills AG window) ----
            with nc.named_scope("transpose"):
                for t in range(NT):
                    for half in range(2):
                        pst = pT.tile([128, 128], f32, tag="tr", bufs=2)
                        nc.tensor.transpose(
                            pst[:],
                            h0_sb[:, t, 128 * half:128 * (half + 1)],
                            ident_v[:],
                        )
                        nc.vector.tensor_copy(
                            hT8[:, half, 128 * t:128 * (t + 1)], pst[:])

            # ---- readback: all-gathered h0 (fp8) into SBUF, per-core chunks
            with nc.named_scope("readback"):
                for gi, (glo, gn) in enumerate(AGS):
                    for r in range(NCORES):
                        nc.sync.dma_start(
                            h0a8[:, r * NT + glo:r * NT + glo + gn, :],
                            ag_outs[gi][r * gn * 128:(r + 1) * gn * 128, :]
                            .rearrange("(t p) m -> p t m", p=128))

            # ---- phase D: SpMM flipped, fp8 DoubleRow ----
            # hX^T[f, d] = sum_src h0[src, f] * A[src, d]; weights = h0 pairs
            with nc.named_scope("spmm"):
                with tc.tile_pool(name="ps", bufs=1, space="PSUM") as ps, \
                     tc.tile_pool(name="pc", bufs=1) as pc:
                    for a, A_d in enumerate([A1, A2]):
                        psS = {}
                        for fh in range(2):
                            for ci, (co, cw) in enumerate(CH):
                                psS[(fh, ci)] = ps.tile(
                                    [128, cw], f32, tag=f"s{fh}{ci}",
                                    name=f"psS{fh}{ci}")
                        for pi, p in enumerate(PAIR_ORDER):
                            a_t = pc.tile([128, 2, PROWS], f8, tag="a",
                                          bufs=24)
                            nc.sync.dma_start(
                                a_t[:],
                                A_d[:, p * 2 * PROWS:(p + 1) * 2 * PROWS]
                                .rearrange("q (two d) -> q two d", two=2))
                            for fh in range(2):
                                for ci, (co, cw) in enumerate(CH):
                                    nc.tensor.matmul(
                                        psS[(fh, ci)][:],
                                        h0a8[:, 2 * p:2 * p + 2,
                                             128 * fh:128 * (fh + 1)],
                                        a_t[:, :, co:co + cw],
                                        start=(pi == 0), stop=(pi == SP - 1),
                                        perf_mode=mybir.MatmulPerfMode.DoubleRow,
                                    )
                        for fh in range(2):
                            for ci, (co, cw) in enumerate(CH):
                                nc.vector.tensor_copy(
                                    hT8[:, 2 + 2 * a + fh, co:co + cw],
                                    psS[(fh, ci)][:])

            # ---- phase E: out = hT @ Wout + b (h0 fp32r, h1/h2 bf16) ----
            with nc.named_scope("out_gemm"), \
                 tc.tile_pool(name="po", bufs=1, space="PSUM") as pO:
                for t in range(NT):
                    psO = pO.tile([128, OUT_C], f32, tag="o", bufs=2)
                    nc.tensor.matmul(psO[:], wo8_sb[0:1, OO8:OO8 + 128],
                                     wo8_sb[0:1, OB8:OB8 + OUT_C],
                                     start=True, stop=False)
                    for k in range(6):
                        nc.tensor.matmul(
                            psO[:],
                            hT8[:, k, 128 * t:128 * (t + 1)],
                            wo8_sb[:, WO8 + k * OUT_C:WO8 + (k + 1) * OUT_C],
                            start=False, stop=(k == 5),
                        )
                    o_sb = keep.tile([128, OUT_C], f32, tag="osb", bufs=2)
                    nc.vector.tensor_copy(o_sb[:], psO[:])
                    rows = min(128, ROWS - 128 * t)
                    nc.sync.dma_start(out[128 * t:128 * t + rows, :],
                                      o_sb[:rows, :])
    nc.compile()
    return nc


def kernel(x, adj1_rows, adj1_cols, adj1_vals, adj2_rows, adj2_cols, adj2_vals,
           W1, W_out, b_out):
    global LAST_EXEC_NS, LAST_RESULTS
    _install_trace_shim()
    x = np.asarray(x, np.float32)
    W1 = np.ascontiguousarray(np.asarray(W1, np.float32))
    W_out = np.ascontiguousarray(np.asarray(W_out, np.float32)).copy()
    b_out = np.asarray(b_out, np.float32).ravel()

    # compensate the fp8 edge-value scaling in W_out rows
    W_out[HID:2 * HID] /= A1_SCALE
    W_out[2 * HID:3 * HID] /= A2_SCALE

    w1_b = W1.reshape(KT, 128, HID).transpose(1, 0, 2).reshape(
        128, KT * HID).astype(bfnp)
    wo8 = np.zeros((128, WO8N), np.float32)
    wo8[:, WO8:WO8 + 6 * OUT_C] = \
        W_out.reshape(6, 128, OUT_C).transpose(1, 0, 2).reshape(128, 6 * OUT_C)
    wo8[0, OB8:OB8 + OUT_C] = b_out
    wo8[0, OO8:OO8 + 128] = 1.0
    wo8 = wo8.astype(bfnp)
    ident = np.eye(128, dtype=np.float32)

    a1r = np.asarray(adj1_rows, np.int64)
    a1c = np.asarray(adj1_cols, np.int64)
    a1v = np.asarray(adj1_vals, np.float32)
    a2r = np.asarray(adj2_rows, np.int64)
    a2c = np.asarray(adj2_cols, np.int64)
    a2v = np.asarray(adj2_vals, np.float32)

    in_maps = []
    for c in range(NCORES):
        xtp = np.zeros((IN_C, PROWS), np.float32)
        xtp[:, :ROWS] = x[c * ROWS:(c + 1) * ROWS].T
        xt_b = xtp.reshape(KT, 128, PROWS).transpose(1, 0, 2).reshape(
            128, KT * PROWS).astype(bfnp)
        in_maps.append({
            "w1": w1_b, "xt": xt_b, "ident": ident, "wo8": wo8,
            "A1": _dense_adj(a1r, a1c, a1v, c, A1_SCALE),
            "A2": _dense_adj(a2r, a2c, a2v, c, A2_SCALE),
        })

    nc = _build()
    try:
        res = bass_utils.run_bass_kernel_spmd(
            nc, in_maps, core_ids=list(range(NCORES)), trace=True,
            trace_cores=[0])
    except Exception:
        res = bass_utils.run_bass_kernel_spmd(
            nc, in_maps, core_ids=list(range(NCORES)), trace=False)
    LAST_EXEC_NS = res.exec_time_ns
    LAST_RESULTS = res
    return np.concatenate([res.results[c]["out"] for c in range(NCORES)], axis=0)


# revision 25
# speedup vs baseline: 1.0119x; 1.0119x over previous
"""H2GCN forward on 8 Trainium2 NeuronCores.

out = concat([h0, A1@h0, A2@h0], 1) @ W_out + b_out,  h0 = x @ W1

Data-parallel over destination nodes (1250 rows/core, padded to 1280).
v3 layout:
  - phase A: h0 = x @ W1 in bf16, k-outer loop over resident xt chunks,
    tiles 0-5 finished first (6 then 4 PSUM accumulators).
  - AllGather of h0 in fp8, split in two (tiles 0-5, 6-9 of every core)
    so SpMM starts on first-half source pairs while the second half is
    still on the wire.  (The collective subsystem has a ~77us boot
    barrier; both AGs queue right behind it.)
  - SpMM flipped: h1^T/h2^T = h0^T A with h0 fp8 pairs as the stationary
    operand (DoubleRow: 256 src rows per matmul) and dense fp8 A^T blocks
    as the moving operand, accumulated over 40 src-tile pairs into 6 PSUM
    banks (2 feature halves x 3 dst chunks).  Edge values pre-scaled
    x16/x32 into fp8 range; compensated in W_out rows.
  - h0 transposes (20) run under the AG window; h1/h2 need no transpose.
  - out = hT @ W_out + b: h0 contribution in fp32r, h1/h2 in bf16.
"""
import sys
import types

for _p in ("/opt/trn_rl_repo", "/root/.axon_site", "/root/.axon_site/_ro/trn_rl_repo",
           "/root/.axon_site/_ro/pypackages"):
    if _p not in sys.path:
        sys.path.append(_p)

import numpy as np
import ml_dtypes
import concourse.bass as bass
import concourse.bacc as bacc
import concourse.mybir as mybir
import concourse.tile as tile
from concourse import bass_utils

N, IN_C, HID, OUT_C = 10000, 2048, 256, 256
NCORES = 8
ROWS = N // NCORES          # 1250
PROWS = 1280                # padded (10 x 128)
NT = PROWS // 128           # 10 dst tiles
KT = IN_C // 128            # 16 k tiles
ST = NCORES * NT            # 80 src tiles in padded AllGather space
SP = ST // 2                # 40 src-tile pairs (DoubleRow)
CH = [(0, 512), (512, 512), (1024, 256)]   # dst chunks within 1280
AGS = [(0, 4), (4, 4), (8, 2)]  # AllGather thirds (tile start, count)

f32 = mybir.dt.float32
f32r = mybir.dt.float32r
bf16 = mybir.dt.bfloat16
f8 = mybir.dt.float8e4
bfnp = ml_dtypes.bfloat16
f8np = ml_dtypes.float8_e4m3

A1_SCALE = 16.0
A2_SCALE = 32.0

# wo8 layout (bf16 elems): Wout k-tiles 0-5 | bias (row 0) | ones (row 0)
WO8, OB8 = 0, 6 * OUT_C
OO8 = OB8 + OUT_C
WO8N = OO8 + 128

# SpMM pair order follows the AllGather thirds: pairs j 0-1 of each core,
# then j 2-3, then j 4
PAIR_ORDER = [r * 5 + j for r in range(NCORES) for j in (0, 1)] + \
             [r * 5 + j for r in range(NCORES) for j in (2, 3)] + \
             [r * 5 + 4 for r in range(NCORES)]

LAST_EXEC_NS = None
LAST_RESULTS = None


def _install_trace_shim():
    try:
        import antenv.axon_hooks  # noqa: F401
        return
    except ImportError:
        pass
    try:
        import antenv
        from trn_agent_boot.trn_boot import _ntff_profile_via_ctypes
        hook = _ntff_profile_via_ctypes("/opt/axon/libaxon_pjrt.so")
        mod = types.ModuleType("antenv.axon_hooks")
        mod.get_axon_ntff_profile_hook = lambda: hook
        mod.set_axon_ntff_profile_hook = lambda h: None
        sys.modules["antenv.axon_hooks"] = mod
        antenv.axon_hooks = mod
    except Exception:
        pass


def _dense_adj(rows, cols, vals, core, scale):
    """Dense padded A^T for this core's dest shard, src-tile-major:
    [128, ST*PROWS] fp8 with src tile s at columns [s*1280, (s+1)*1280)."""
    lo, hi = core * ROWS, (core + 1) * ROWS
    m = (rows >= lo) & (rows < hi)
    r, c, v = rows[m] - lo, cols[m], vals[m] * scale
    A = np.zeros((NCORES * PROWS, PROWS), np.float32)
    src = (c // ROWS) * PROWS + (c % ROWS)
    np.add.at(A, (src, r), v)
    return np.ascontiguousarray(
        A.reshape(ST, 128, PROWS).transpose(1, 0, 2)
        .reshape(128, ST * PROWS)).astype(f8np)


def _build():
    nc = bacc.Bacc("TRN2", target_bir_lowering=False, debug=False,
                   num_devices=8)
    w1_d = nc.dram_tensor("w1", [128, KT * HID], bf16, kind="ExternalInput")
    xt_d = nc.dram_tensor("xt", [128, KT * PROWS], bf16, kind="ExternalInput")
    ident_d = nc.dram_tensor("ident", [128, 128], f32, kind="ExternalInput")
    wo8_d = nc.dram_tensor("wo8", [128, WO8N], bf16, kind="ExternalInput")
    A1 = nc.dram_tensor("A1", [128, ST * PROWS], f8, kind="ExternalInput")
    A2 = nc.dram_tensor("A2", [128, ST * PROWS], f8, kind="ExternalInput")
    out = nc.dram_tensor("out", [ROWS, OUT_C], f32, kind="ExternalOutput")

    with tile.TileContext(nc) as tc:
        with tc.tile_pool(name="keep", bufs=1) as keep, \
             tc.tile_pool(name="dram", bufs=1, space="DRAM") as dram, \
             tc.tile_pool(name="pT", bufs=1, space="PSUM") as pT:

            h0_sb = keep.tile([128, NT, HID], f32)
            ag_sb = keep.tile([128, NT, HID], f8)
            h0a8 = keep.tile([128, ST, HID], f8)
            hT8 = keep.tile([128, 6, PROWS], bf16)
            wo8_sb = keep.tile([128, WO8N], bf16)
            ident_v = keep.tile([128, 128], f32)
            w1_sb = keep.tile([128, KT, HID], bf16)
            nc.sync.dma_start(w1_sb[:], w1_d[:].rearrange(
                "p (k m) -> p k m", k=KT))
            nc.sync.dma_start(wo8_sb[:], wo8_d[:])
            ident_t = keep.tile([128, 128], f32)
            nc.sync.dma_start(ident_t[:], ident_d[:])
            # identity produced on DVE so transposes need only one DVE wait
            nc.vector.tensor_copy(ident_v[:], ident_t[:])

            ag_ins, ag_outs = [], []
            for gi, (glo, gn) in enumerate(AGS):
                ag_ins.append(dram.tile([gn * 128, HID], f8,
                                        name=f"ag_in{gi}"))
                ag_outs.append(dram.tile([NCORES * gn * 128, HID], f8,
                                         addr_space="Shared",
                                         name=f"ag_out{gi}"))

            # ---- phase A: h0 = x @ W1 (bf16), k-outer so DMA pipelines.
            # Tiles 0-5 first so the first AllGather half can launch early.
            with nc.named_scope("h0_gemm"):
                with tc.tile_pool(name="pa", bufs=1, space="PSUM") as pa, \
                     tc.tile_pool(name="px", bufs=1) as px:
                    xts = []
                    for k in range(KT):
                        xt_k = px.tile([128, PROWS], bf16, tag=f"xt{k}",
                                       name=f"xt{k}")
                        nc.sync.dma_start(xt_k[:],
                                          xt_d[:, k * PROWS:(k + 1) * PROWS])
                        xts.append(xt_k)
                    for tlo, tn in ((0, 5), (5, 5)):
                        psA = [pa.tile([128, HID], f32, tag=f"a{i}",
                                       name=f"psA{i}") for i in range(tn)]
                        for k in range(KT):
                            for i in range(tn):
                                t = tlo + i
                                nc.tensor.matmul(
                                    psA[i][:],
                                    xts[k][:, 128 * t:128 * (t + 1)],
                                    w1_sb[:, k, :],
                                    start=(k == 0), stop=(k == KT - 1),
                                )
                        for i in range(tn):
                            t = tlo + i
                            nc.vector.tensor_copy(h0_sb[:, t, :], psA[i][:])
                            nc.vector.tensor_copy(ag_sb[:, t, :], psA[i][:])
                    for gi, (glo, gn) in enumerate(AGS):
                        nc.sync.dma_start(
                            ag_ins[gi][:].rearrange("(a p) m -> p a m", p=128),
                            ag_sb[:, glo:glo + gn, :])

            # ---- phase B: AllGather h0 (fp8), three chunks ----
            with nc.named_scope("allgather"):
                for gi in range(len(AGS)):
                    nc.gpsimd.collective_compute(
                        "AllGather", mybir.AluOpType.bypass,
                        replica_groups=[list(range(NCORES))],
                        ins=[ag_ins[gi].opt()], outs=[ag_outs[gi].opt()],
                    )

            # ---- phase C: transpose h0 -> feature-major (fills AG window) ----
            with nc.named_scope("transpose"):
                for t in range(NT):
                    for half in range(2):
                        pst = pT.tile([128, 128], f32, tag="tr", bufs=2)
                        nc.tensor.transpose(
                            pst[:],
                            h0_sb[:, t, 128 * half:128 * (half + 1)],
                            ident_v[:],
                        )
                        nc.vector.tensor_copy(
                            hT8[:, half, 128 * t:128 * (t + 1)], pst[:])

            # ---- readback: all-gathered h0 (fp8) into SBUF, per-core chunks
            with nc.named_scope("readback"):
                for gi, (glo, gn) in enumerate(AGS):
                    for r in range(NCORES):
                        nc.sync.dma_start(
                            h0a8[:, r * NT + glo:r * NT + glo + gn, :],
                            ag_outs[gi][r * gn * 128:(r + 1) * gn * 128, :]
                            .rearrange("(t p) m -> p t m", p=128))

            # ---- phase D: SpMM flipped, fp8 DoubleRow ----
            # hX^T[f, d] = sum_src h0[src, f] * A[src, d]; weights = h0 pairs
            with nc.named_scope("spmm"):
                with tc.tile_pool(name="ps", bufs=1, space="PSUM") as ps, \
                     tc.tile_pool(name="pc", bufs=1) as pc:
                    for a, A_d in enumerate([A1, A2]):
                        psS = {}
                        for fh in range(2):
                            for ci, (co, cw) in enumerate(CH):
                                psS[(fh, ci)] = ps.tile(
                                    [128, cw], f32, tag=f"s{fh}{ci}",
                                    name=f"psS{fh}{ci}")
                        for pi, p in enumerate(PAIR_ORDER):
                            a_t = pc.tile([128, 2, PROWS], f8, tag="a",
                                          bufs=24)
                            nc.sync.dma_start(
                                a_t[:],
                                A_d[:, p * 2 * PROWS:(p + 1) * 2 * PROWS]
                                .rearrange("q (two d) -> q two d", two=2))
                            for fh in range(2):
                                for ci, (co, cw) in enumerate(CH):
                                    nc.tensor.matmul(
                                        psS[(fh, ci)][:],
                                        h0a8[:, 2 * p:2 * p + 2,
                                             128 * fh:128 * (fh + 1)],
                                        a_t[:, :, co:co + cw],
                                        start=(pi == 0), stop=(pi == SP - 1),
                                        perf_mode=mybir.MatmulPerfMode.DoubleRow,
                                    )
                        for fh in range(2):
                            for ci, (co, cw) in enumerate(CH):
                                nc.vector.tensor_copy(
                                    hT8[:, 2 + 2 * a + fh, co:co + cw],
                                    psS[(fh, ci)][:])

            # ---- phase E: out = hT @ Wout + b (h0 fp32r, h1/h2 bf16) ----
            with nc.named_scope("out_gemm"), \
                 tc.tile_pool(name="po", bufs=1, space="PSUM") as pO:
                for t in range(NT):
                    psO = pO.tile([128, OUT_C], f32, tag="o", bufs=2)
                    nc.tensor.matmul(psO[:], wo8_sb[0:1, OO8:OO8 + 128],
                                     wo8_sb[0:1, OB8:OB8 + OUT_C],
                                     start=True, stop=False)
                    for k in range(6):
                        nc.tensor.matmul(
                            psO[:],
                            hT8[:, k, 128 * t:128 * (t + 1)],
                            wo8_sb[:, WO8 + k * OUT_C:WO8 + (k + 1) * OUT_C],
                            start=False, stop=(k == 5),
                        )
                    o_sb = keep.tile([128, OUT_C], f32, tag="osb", bufs=2)
                    nc.vector.tensor_copy(o_sb[:], psO[:])
                    rows = min(128, ROWS - 128 * t)
                    nc.sync.dma_start(out[128 * t:128 * t + rows, :],
                                      o_sb[:rows, :])
    nc.compile()
    return nc


def kernel(x, adj1_rows, adj1_cols, adj1_vals, adj2_rows, adj2_cols, adj2_vals,
           W1, W_out, b_out):
    global LAST_EXEC_NS, LAST_RESULTS
    _install_trace_shim()
    x = np.asarray(x, np.float32)
    W1 = np.ascontiguousarray(np.asarray(W1, np.float32))
    W_out = np.ascontiguousarray(np.asarray(W_out, np.float32)).copy()
    b_out = np.asarray(b_out, np.float32).ravel()

    # compensate the fp8 edge-value scaling in W_out rows
    W_out[HID:2 * HID] /= A1_SCALE
    W_out[2 * HID:3 * HID] /= A2_SCALE

    w1_b = W1.reshape(KT, 128, HID).transpose(1, 0, 2).reshape(
        128, KT * HID).astype(bfnp)
    wo8 = np.zeros((128, WO8N), np.float32)
    wo8[:, WO8:WO8 + 6 * OUT_C] = \
        W_out.reshape(6, 128, OUT_C).transpose(1, 0, 2).reshape(128, 6 * OUT_C)
    wo8[0, OB8:OB8 + OUT_C] = b_out
    wo8[0, OO8:OO8 + 128] = 1.0
    wo8 = wo8.astype(bfnp)
    ident = np.eye(128, dtype=np.float32)

    a1r = np.asarray(adj1_rows, np.int64)
    a1c = np.asarray(adj1_cols, np.int64)
    a1v = np.asarray(adj1_vals, np.float32)
    a2r = np.asarray(adj2_rows, np.int64)
    a2c = np.asarray(adj2_cols, np.int64)
    a2v = np.asarray(adj2_vals, np.float32)

    in_maps = []
    for c in range(NCORES):
        xtp = np.zeros((IN_C, PROWS), np.float32)
        xtp[:, :ROWS] = x[c * ROWS:(c + 1) * ROWS].T
        xt_b = xtp.reshape(KT, 128, PROWS).transpose(1, 0, 2).reshape(
            128, KT * PROWS).astype(bfnp)
        in_maps.append({
            "w1": w1_b, "xt": xt_b, "ident": ident, "wo8": wo8,
            "A1": _dense_adj(a1r, a1c, a1v, c, A1_SCALE),
            "A2": _dense_adj(a2r, a2c, a2v, c, A2_SCALE),
        })

    nc = _build()
    try:
        res = bass_utils.run_bass_kernel_spmd(
            nc, in_maps, core_ids=list(range(NCORES)), trace=True,
            trace_cores=[0])
    except Exception:
        res = bass_utils.run_bass_kernel_spmd(
            nc, in_maps, core_ids=list(range(NCORES)), trace=False)
    LAST_EXEC_NS = res.exec_time_ns
    LAST_RESULTS = res
    return np.concatenate([res.results[c]["out"] for c in range(NCORES)], axis=0)


# revision 34
# speedup vs baseline: 1.0926x; 1.0798x over previous
"""H2GCN forward on 8 Trainium2 NeuronCores.

out = concat([h0, A1@h0, A2@h0], 1) @ W_out + b_out,  h0 = x @ W1

Data-parallel over destination nodes (1250 rows/core, padded to 1280).
v3 layout:
  - phase A: h0 = x @ W1 in bf16, k-outer loop over resident xt chunks,
    tiles 0-5 finished first (6 then 4 PSUM accumulators).
  - AllGather of h0 in fp8, split in two (tiles 0-5, 6-9 of every core)
    so SpMM starts on first-half source pairs while the second half is
    still on the wire.  (The collective subsystem has a ~77us boot
    barrier; both AGs queue right behind it.)
  - SpMM flipped: h1^T/h2^T = h0^T A with h0 fp8 pairs as the stationary
    operand (DoubleRow: 256 src rows per matmul) and dense fp8 A^T blocks
    as the moving operand, accumulated over 40 src-tile pairs into 6 PSUM
    banks (2 feature halves x 3 dst chunks).  Edge values pre-scaled
    x16/x32 into fp8 range; compensated in W_out rows.
  - h0 transposes (20) run under the AG window; h1/h2 need no transpose.
  - out = hT @ W_out + b: h0 contribution in fp32r, h1/h2 in bf16.
"""
import sys
import types

for _p in ("/opt/trn_rl_repo", "/root/.axon_site", "/root/.axon_site/_ro/trn_rl_repo",
           "/root/.axon_site/_ro/pypackages"):
    if _p not in sys.path:
        sys.path.append(_p)

import numpy as np
import ml_dtypes
import concourse.bass as bass
import concourse.bacc as bacc
import concourse.mybir as mybir
import concourse.tile as tile
from concourse import bass_utils

N, IN_C, HID, OUT_C = 10000, 2048, 256, 256
NCORES = 8
ROWS = N // NCORES          # 1250
PROWS = 1280                # padded (10 x 128)
NT = PROWS // 128           # 10 dst tiles
KT = IN_C // 128            # 16 k tiles
ST = NCORES * NT            # 80 src tiles in padded AllGather space
SP = ST // 2                # 40 src-tile pairs (DoubleRow)
CH = [(0, 512), (512, 512), (1024, 256)]   # dst chunks within 1280
AGS = [(0, 8), (8, 2)]          # AllGather halves (tile start, count)

f32 = mybir.dt.float32
f32r = mybir.dt.float32r
bf16 = mybir.dt.bfloat16
f8 = mybir.dt.float8e4
bfnp = ml_dtypes.bfloat16
f8np = ml_dtypes.float8_e4m3

A1_SCALE = 16.0
A2_SCALE = 32.0

# wo8 layout (bf16 elems): Wout k-tiles 0-5 | bias (row 0) | ones (row 0)
WO8, OB8 = 0, 6 * OUT_C
OO8 = OB8 + OUT_C
WO8N = OO8 + 128

# SpMM pair order follows the AllGather halves: pairs j 0-3 of each core,
# then j 4
PAIR_ORDER = [r * 5 + j for r in range(NCORES) for j in (0, 1, 2, 3)] + \
             [r * 5 + 4 for r in range(NCORES)]

LAST_EXEC_NS = None
LAST_RESULTS = None


def _install_trace_shim():
    try:
        import antenv.axon_hooks  # noqa: F401
        return
    except ImportError:
        pass
    try:
        import antenv
        from trn_agent_boot.trn_boot import _ntff_profile_via_ctypes
        hook = _ntff_profile_via_ctypes("/opt/axon/libaxon_pjrt.so")
        mod = types.ModuleType("antenv.axon_hooks")
        mod.get_axon_ntff_profile_hook = lambda: hook
        mod.set_axon_ntff_profile_hook = lambda h: None
        sys.modules["antenv.axon_hooks"] = mod
        antenv.axon_hooks = mod
    except Exception:
        pass


def _dense_adj(rows, cols, vals, core, scale):
    """Dense padded A^T for this core's dest shard, src-tile-major:
    [128, ST*PROWS] fp8 with src tile s at columns [s*1280, (s+1)*1280)."""
    lo, hi = core * ROWS, (core + 1) * ROWS
    m = (rows >= lo) & (rows < hi)
    r, c, v = rows[m] - lo, cols[m], vals[m] * scale
    A = np.zeros((NCORES * PROWS, PROWS), np.float32)
    src = (c // ROWS) * PROWS + (c % ROWS)
    np.add.at(A, (src, r), v)
    return np.ascontiguousarray(
        A.reshape(ST, 128, PROWS).transpose(1, 0, 2)
        .reshape(128, ST * PROWS)).astype(f8np)


def _build():
    nc = bacc.Bacc("TRN2", target_bir_lowering=False, debug=False,
                   num_devices=8)
    w1_d = nc.dram_tensor("w1", [128, KT * HID], bf16, kind="ExternalInput")
    xt_d = nc.dram_tensor("xt", [128, KT * PROWS], bf16, kind="ExternalInput")
    ident_d = nc.dram_tensor("ident", [128, 128], f32, kind="ExternalInput")
    wo8_d = nc.dram_tensor("wo8", [128, WO8N], bf16, kind="ExternalInput")
    A1 = nc.dram_tensor("A1", [128, ST * PROWS], f8, kind="ExternalInput")
    A2 = nc.dram_tensor("A2", [128, ST * PROWS], f8, kind="ExternalInput")
    out = nc.dram_tensor("out", [ROWS, OUT_C], f32, kind="ExternalOutput")

    with tile.TileContext(nc) as tc:
        with tc.tile_pool(name="keep", bufs=1) as keep, \
             tc.tile_pool(name="dram", bufs=1, space="DRAM") as dram, \
             tc.tile_pool(name="pT", bufs=1, space="PSUM") as pT:

            h0_sb = keep.tile([128, NT, HID], f32)
            ag_sb = keep.tile([128, NT, HID], f8)
            h0a8 = keep.tile([128, ST, HID], f8)
            hT8 = keep.tile([128, 6, PROWS], bf16)
            wo8_sb = keep.tile([128, WO8N], bf16)
            ident_v = keep.tile([128, 128], f32)
            w1_sb = keep.tile([128, KT, HID], bf16)
            nc.sync.dma_start(w1_sb[:], w1_d[:].rearrange(
                "p (k m) -> p k m", k=KT))
            nc.sync.dma_start(wo8_sb[:], wo8_d[:])
            ident_t = keep.tile([128, 128], f32)
            nc.sync.dma_start(ident_t[:], ident_d[:])
            # identity produced on DVE so transposes need only one DVE wait
            nc.vector.tensor_copy(ident_v[:], ident_t[:])

            ag_ins, ag_outs = [], []
            for gi, (glo, gn) in enumerate(AGS):
                ag_ins.append(dram.tile([gn * 128, HID], f8,
                                        name=f"ag_in{gi}"))
                ag_outs.append(dram.tile([NCORES * gn * 128, HID], f8,
                                         addr_space="Shared",
                                         name=f"ag_out{gi}"))

            # ---- phase A: h0 = x @ W1 (bf16), k-outer so DMA pipelines.
            # Tiles 0-5 first so the first AllGather half can launch early.
            with nc.named_scope("h0_gemm"):
                with tc.tile_pool(name="pa", bufs=1, space="PSUM") as pa, \
                     tc.tile_pool(name="px", bufs=1) as px:
                    xts = []
                    for k in range(KT):
                        xt_k = px.tile([128, PROWS], bf16, tag=f"xt{k}",
                                       name=f"xt{k}")
                        nc.sync.dma_start(xt_k[:],
                                          xt_d[:, k * PROWS:(k + 1) * PROWS])
                        xts.append(xt_k)
                    for tlo, tn in ((0, 5), (5, 5)):
                        psA = [pa.tile([128, HID], f32, tag=f"a{i}",
                                       name=f"psA{i}") for i in range(tn)]
                        for k in range(KT):
                            for i in range(tn):
                                t = tlo + i
                                nc.tensor.matmul(
                                    psA[i][:],
                                    xts[k][:, 128 * t:128 * (t + 1)],
                                    w1_sb[:, k, :],
                                    start=(k == 0), stop=(k == KT - 1),
                                )
                        for i in range(tn):
                            t = tlo + i
                            nc.vector.tensor_copy(h0_sb[:, t, :], psA[i][:])
                            nc.vector.tensor_copy(ag_sb[:, t, :], psA[i][:])
                    for gi, (glo, gn) in enumerate(AGS):
                        nc.sync.dma_start(
                            ag_ins[gi][:].rearrange("(a p) m -> p a m", p=128),
                            ag_sb[:, glo:glo + gn, :])

            # ---- phase B: AllGather h0 (fp8), three chunks ----
            with nc.named_scope("allgather"):
                for gi in range(len(AGS)):
                    nc.gpsimd.collective_compute(
                        "AllGather", mybir.AluOpType.bypass,
                        replica_groups=[list(range(NCORES))],
                        ins=[ag_ins[gi].opt()], outs=[ag_outs[gi].opt()],
                    )

            # ---- phase C: transpose h0 -> feature-major (fills AG window) ----
            with nc.named_scope("transpose"):
                for t in range(NT):
                    for half in range(2):
                        pst = pT.tile([128, 128], f32, tag="tr", bufs=2)
                        nc.tensor.transpose(
                            pst[:],
                            h0_sb[:, t, 128 * half:128 * (half + 1)],
                            ident_v[:],
                        )
                        nc.vector.tensor_copy(
                            hT8[:, half, 128 * t:128 * (t + 1)], pst[:])

            # ---- prefold: bias + h0 @ Wout[k0,k1] into o_part (fills the
            # collective window; the tail only adds the h1/h2 terms)
            o_part = keep.tile([128, NT, OUT_C], f32)
            with nc.named_scope("prefold"):
                with tc.tile_pool(name="pp", bufs=1, space="PSUM") as pp:
                    for tlo in (0, 5):
                        psP = [pp.tile([128, OUT_C], f32, tag=f"p{i}",
                                       name=f"psP{i}") for i in range(5)]
                        for i in range(5):
                            t = tlo + i
                            nc.tensor.matmul(psP[i][:],
                                             wo8_sb[0:1, OO8:OO8 + 128],
                                             wo8_sb[0:1, OB8:OB8 + OUT_C],
                                             start=True, stop=False)
                            for k in range(2):
                                nc.tensor.matmul(
                                    psP[i][:],
                                    hT8[:, k, 128 * t:128 * (t + 1)],
                                    wo8_sb[:, WO8 + k * OUT_C:
                                           WO8 + (k + 1) * OUT_C],
                                    start=False, stop=(k == 1),
                                )
                        for i in range(5):
                            nc.vector.tensor_copy(o_part[:, tlo + i, :],
                                                  psP[i][:])

            # ---- readback: all-gathered h0 (fp8) into SBUF, one DMA per
            # source pair, issued in SpMM consumption order
            with nc.named_scope("readback"):
                for gi, (glo, gn) in enumerate(AGS):
                    for r in range(NCORES):
                        for j in range(gn // 2):
                            t0 = glo + 2 * j
                            ro = r * gn * 128 + j * 256
                            nc.sync.dma_start(
                                h0a8[:, r * NT + t0:r * NT + t0 + 2, :],
                                ag_outs[gi][ro:ro + 256, :]
                                .rearrange("(t p) m -> p t m", p=128))

            # ---- phase D: SpMM flipped, fp8 DoubleRow ----
            # hX^T[f, d] = sum_src h0[src, f] * A[src, d]; weights = h0 pairs
            with nc.named_scope("spmm"):
                with tc.tile_pool(name="ps", bufs=1, space="PSUM") as ps, \
                     tc.tile_pool(name="pc", bufs=1) as pc:
                    for a, A_d in enumerate([A1, A2]):
                        psS = {}
                        for fh in range(2):
                            for ci, (co, cw) in enumerate(CH):
                                psS[(fh, ci)] = ps.tile(
                                    [128, cw], f32, tag=f"s{fh}{ci}",
                                    name=f"psS{fh}{ci}")
                        for pi, p in enumerate(PAIR_ORDER):
                            a_t = pc.tile([128, 2, PROWS], f8, tag="a",
                                          bufs=24)
                            nc.sync.dma_start(
                                a_t[:],
                                A_d[:, p * 2 * PROWS:(p + 1) * 2 * PROWS]
                                .rearrange("q (two d) -> q two d", two=2))
                            for fh in range(2):
                                for ci, (co, cw) in enumerate(CH):
                                    nc.tensor.matmul(
                                        psS[(fh, ci)][:],
                                        h0a8[:, 2 * p:2 * p + 2,
                                             128 * fh:128 * (fh + 1)],
                                        a_t[:, :, co:co + cw],
                                        start=(pi == 0), stop=(pi == SP - 1),
                                        perf_mode=mybir.MatmulPerfMode.DoubleRow,
                                    )
                        for fh in range(2):
                            for ci, (co, cw) in enumerate(CH):
                                nc.vector.tensor_copy(
                                    hT8[:, 2 + 2 * a + fh, co:co + cw],
                                    psS[(fh, ci)][:])

            # ---- phase E: out = hT @ Wout + b (h0 fp32r, h1/h2 bf16) ----
            with nc.named_scope("out_gemm"), \
                 tc.tile_pool(name="po", bufs=1, space="PSUM") as pO:
                for t in range(NT):
                    psO = pO.tile([128, OUT_C], f32, tag="o", bufs=2)
                    for k in range(2, 6):
                        nc.tensor.matmul(
                            psO[:],
                            hT8[:, k, 128 * t:128 * (t + 1)],
                            wo8_sb[:, WO8 + k * OUT_C:WO8 + (k + 1) * OUT_C],
                            start=(k == 2), stop=(k == 5),
                        )
                    o_sb = keep.tile([128, OUT_C], f32, tag="osb", bufs=2)
                    nc.vector.scalar_tensor_tensor(
                        o_sb[:], psO[:], 1.0, o_part[:, t, :],
                        mybir.AluOpType.mult, mybir.AluOpType.add)
                    rows = min(128, ROWS - 128 * t)
                    nc.sync.dma_start(out[128 * t:128 * t + rows, :],
                                      o_sb[:rows, :])
    nc.compile()
    return nc


def kernel(x, adj1_rows, adj1_cols, adj1_vals, adj2_rows, adj2_cols, adj2_vals,
           W1, W_out, b_out):
    global LAST_EXEC_NS, LAST_RESULTS
    _install_trace_shim()
    x = np.asarray(x, np.float32)
    W1 = np.ascontiguousarray(np.asarray(W1, np.float32))
    W_out = np.ascontiguousarray(np.asarray(W_out, np.float32)).copy()
    b_out = np.asarray(b_out, np.float32).ravel()

    # compensate the fp8 edge-value scaling in W_out rows
    W_out[HID:2 * HID] /= A1_SCALE
    W_out[2 * HID:3 * HID] /= A2_SCALE

    w1_b = W1.reshape(KT, 128, HID).transpose(1, 0, 2).reshape(
        128, KT * HID).astype(bfnp)
    wo8 = np.zeros((128, WO8N), np.float32)
    wo8[:, WO8:WO8 + 6 * OUT_C] = \
        W_out.reshape(6, 128, OUT_C).transpose(1, 0, 2).reshape(128, 6 * OUT_C)
    wo8[0, OB8:OB8 + OUT_C] = b_out
    wo8[0, OO8:OO8 + 128] = 1.0
    wo8 = wo8.astype(bfnp)
    ident = np.eye(128, dtype=np.float32)

    a1r = np.asarray(adj1_rows, np.int64)
    a1c = np.asarray(adj1_cols, np.int64)
    a1v = np.asarray(adj1_vals, np.float32)
    a2r = np.asarray(adj2_rows, np.int64)
    a2c = np.asarray(adj2_cols, np.int64)
    a2v = np.asarray(adj2_vals, np.float32)

    in_maps = []
    for c in range(NCORES):
        xtp = np.zeros((IN_C, PROWS), np.float32)
        xtp[:, :ROWS] = x[c * ROWS:(c + 1) * ROWS].T
        xt_b = xtp.reshape(KT, 128, PROWS).transpose(1, 0, 2).reshape(
            128, KT * PROWS).astype(bfnp)
        in_maps.append({
            "w1": w1_b, "xt": xt_b, "ident": ident, "wo8": wo8,
            "A1": _dense_adj(a1r, a1c, a1v, c, A1_SCALE),
            "A2": _dense_adj(a2r, a2c, a2v, c, A2_SCALE),
        })

    nc = _build()
    try:
        res = bass_utils.run_bass_kernel_spmd(
            nc, in_maps, core_ids=list(range(NCORES)), trace=True,
            trace_cores=[0])
    except Exception:
        res = bass_utils.run_bass_kernel_spmd(
            nc, in_maps, core_ids=list(range(NCORES)), trace=False)
    LAST_EXEC_NS = res.exec_time_ns
    LAST_RESULTS = res
    return np.concatenate([res.results[c]["out"] for c in range(NCORES)], axis=0)
